# revision 1
# baseline (speedup 1.0000x reference)
"""HADES (selective-filter Mamba2-style SSM) kernel for 8 Trainium2 cores.

Strategy: the sequential/selection parts (cumsum residual, top-k filter
selection, depthwise conv, SSM scan, gated RMSNorm) are computed exactly in
numpy; the final dense out_proj matmul (17 GFLOP, the largest single dense op)
runs as an SPMD Bass/Tile kernel sharded token-parallel across the 8
NeuronCores. If the device path fails for any reason we fall back to numpy so
the returned output is always correct.
"""

import numpy as np

D_MODEL = 1024
D_STATE = 128
D_CONV = 4
HEADDIM = 64
NUM_FILTERS = 16
SHARED_FILTERS = 4
TOTAL_FILTERS = 32
SELECT_FILTERS = 28
N_SELECT = 12
D_INNER = 1024
D_SSM = 1024
CONV_DIM = 1280
EPS = 1e-5


def _softplus(x):
    return np.logaddexp(0.0, x)


def _silu(x):
    return x / (1.0 + np.exp(-x))


def _bass_out_proj(y2d, w):
    """y2d: [T,1024] f32, w: [1024 out,1024 in] f32 -> [T,1024] via 8-core Bass."""
    import concourse.bass as bass
    import concourse.mybir as mybir
    import concourse.tile as tile
    from concourse.bass_utils import run_bass_kernel_spmd

    T, D = y2d.shape
    E = w.shape[0]
    n_cores = 8
    tpc = T // n_cores  # 1024 tokens per core
    KB, EB, NT = D // 128, E // 128, 512
    NN = tpc // NT

    nc = bass.Bass()
    yT_in = nc.declare_dram_parameter("yT", [D, tpc], mybir.dt.float32, isOutput=False)
    wT_in = nc.declare_dram_parameter("wT", [D, E], mybir.dt.float32, isOutput=False)
    outT = nc.declare_dram_parameter("outT", [E, tpc], mybir.dt.float32, isOutput=True)

    with tile.TileContext(nc) as tc:
        with (
            tc.tile_pool(name="w", bufs=1) as wp,
            tc.tile_pool(name="x", bufs=2) as xp,
            tc.tile_pool(name="o", bufs=4) as op_,
            tc.tile_pool(name="ps", bufs=4, space="PSUM") as pp,
        ):
            wt = wp.tile([128, KB, E], mybir.dt.float32)
            nc.sync.dma_start(wt[:], wT_in.rearrange("(k p) e -> p k e", p=128))
            for nb in range(NN):
                xt = xp.tile([128, KB, NT], mybir.dt.float32)
                nc.sync.dma_start(
                    xt[:],
                    yT_in[:, nb * NT:(nb + 1) * NT].rearrange("(k p) n -> p k n", p=128),
                )
                for eb in range(EB):
                    acc = pp.tile([128, NT], mybir.dt.float32)
                    for kb in range(KB):
                        nc.tensor.matmul(
                            acc[:],
                            wt[:, kb, eb * 128:(eb + 1) * 128],
                            xt[:, kb, :],
                            start=(kb == 0),
                            stop=(kb == KB - 1),
                        )
                    ot = op_.tile([128, NT], mybir.dt.float32)
                    nc.scalar.copy(ot[:], acc[:])
                    nc.sync.dma_start(
                        outT[eb * 128:(eb + 1) * 128, nb * NT:(nb + 1) * NT], ot[:]
                    )

    wT = np.ascontiguousarray(w.T.astype(np.float32))
    in_maps = [
        {"yT": np.ascontiguousarray(y2d[c * tpc:(c + 1) * tpc].T), "wT": wT}
        for c in range(n_cores)
    ]
    res = run_bass_kernel_spmd(nc, in_maps, list(range(n_cores)))
    outs = [np.asarray(r["outT"]).T for r in res.results]
    return np.concatenate(outs, 0)


def kernel(u, in_proj_w, h_proj_w, conv_w, conv_b, dt_bias, gamma,
           A_log, D, norm_w, out_proj_w):
    u = np.asarray(u, np.float32)
    b, l, _ = u.shape

    zxbcdt = u @ np.asarray(in_proj_w, np.float32).T          # [B,L,2336]
    dt_all = zxbcdt[..., -TOTAL_FILTERS:]                      # [B,L,32]

    denom = np.arange(1, l + 1, dtype=np.float64)[None, :, None]
    spectral_residual = (u - np.cumsum(u.astype(np.float64), 1) / denom).astype(np.float32)

    udt = np.concatenate([spectral_residual, dt_all], -1)      # [B,L,1056]
    hb = udt @ np.asarray(h_proj_w, np.float32).T              # [B,L,40]
    h = hb[..., :SELECT_FILTERS]
    spectral_bias = hb[..., SELECT_FILTERS:]

    # top-k (descending, ties -> lower index, matching lax.top_k)
    select_ids = np.argsort(-h, axis=-1, kind="stable")[..., :N_SELECT]
    shared_ids = np.broadcast_to(
        np.arange(SHARED_FILTERS, dtype=select_ids.dtype) + SELECT_FILTERS,
        (b, l, SHARED_FILTERS))
    ids = np.concatenate([select_ids, shared_ids], -1)         # [B,L,16]
    dt = np.take_along_axis(dt_all, ids, -1)
    sb = np.asarray(gamma, np.float32) * np.concatenate(
        [spectral_bias, np.zeros((b, l, SHARED_FILTERS), np.float32)], -1)
    dt = dt + np.asarray(dt_bias, np.float32) + sb

    z = zxbcdt[..., :D_SSM]
    xBC = zxbcdt[..., D_SSM:D_SSM + CONV_DIM]                  # [B,L,1280]

    # depthwise causal conv1d + silu
    cw = np.asarray(conv_w, np.float32)                        # [1280,4]
    conv = np.zeros_like(xBC)
    for k in range(D_CONV):
        shift = D_CONV - 1 - k
        src = xBC[:, :l - shift] if shift else xBC
        conv[:, shift:] += src * cw[None, None, :, k]
    xBC_c = _silu(conv + np.asarray(conv_b, np.float32)[None, None, :])

    x = xBC_c[..., :D_SSM].reshape(b, l, NUM_FILTERS, HEADDIM)
    Bm = xBC_c[..., D_SSM:D_SSM + D_STATE]
    Cm = xBC_c[..., D_SSM + D_STATE:]

    dt = _softplus(dt).astype(np.float32)
    A = -np.exp(np.asarray(A_log, np.float32))
    decay = np.exp(dt * A[None, None, :])                      # [B,L,16]

    dtx = dt[..., None] * x                                    # [B,L,16,64]
    state = np.zeros((b, NUM_FILTERS, HEADDIM, D_STATE), np.float32)
    y = np.empty((b, l, NUM_FILTERS, HEADDIM), np.float32)
    for t in range(l):
        state *= decay[:, t, :, None, None]
        state += dtx[:, t, :, :, None] * Bm[:, t, None, None, :]
        y[:, t] = np.einsum("bhpn,bn->bhp", state, Cm[:, t])
    y = y + np.asarray(D, np.float32)[None, None, :, None] * x
    y = y.reshape(b, l, D_SSM)

    yz = y * _silu(z)
    y = yz * (1.0 / np.sqrt(np.mean(yz * yz, -1, keepdims=True) + EPS)) \
        * np.asarray(norm_w, np.float32)

    y2d = np.ascontiguousarray(y.reshape(b * l, D_SSM), np.float32)
    w = np.asarray(out_proj_w, np.float32)
    try:
        out = _bass_out_proj(y2d, w)
    except Exception as e:  # device unavailable -> numpy fallback
        import sys
        print(f"[kernel] bass out_proj failed ({e!r}); numpy fallback", file=sys.stderr)
        out = y2d @ w.T
    return out.reshape(b, l, D_MODEL).astype(np.float32)



# revision 2
# speedup vs baseline: 249.9467x; 249.9467x over previous
"""HADES (selective-filter Mamba2-style SSM) — full on-device Bass kernel, 8 cores.

Sharding: core c -> (batch b = c//2, head-group g = c%2, heads 8g..8g+8,
out_proj output dims 512g..512g+512).

Device pipeline per core:
  in_proj (bf16 matmuls, channel-major out) -> depthwise causal conv+silu ->
  chunked SSD scan (Q=128 chunks: intra-chunk via masked decay matmul,
  inter-chunk state recurrence) -> gated y*silu(z) -> pairwise AllGather of
  yz (full 1024 channels) -> RMSNorm (rsqrt folded into out_proj evac,
  norm_w folded into out_proj weights) -> out_proj (512 out dims per core).

Host does only the tiny data-dependent dt path (top-k filter selection),
which commutes with the cumsum: h = p40 - cummean(p40) + dt_all @ W2^T with
p40 = u @ W1^T. Host also precomputes per-chunk decay vectors (O(L*H)).
"""

import numpy as np
import ml_dtypes

BF16 = ml_dtypes.bfloat16
F32 = np.float32

B, L, DM = 4, 2048, 1024
H, P, N, Q = 16, 64, 128, 128
NCH = L // Q          # 16 chunks
HL = 8                # heads per core
EPS = 1e-5
NEG = -1e30

# f32 const pack column offsets (cf [128, CF])
CF_CW = 0             # [128, 6, 4] conv weights      -> cols 0..24
CF_CB = 24            # [128, 6]    conv bias         -> 24..30
CF_WPP = 30           # [128, 8, 16] w_j              -> 30..158
CF_DLPP = 158         # [128, 8, 16] decay_last       -> 158..286
CF_DPP = 286          # [128, 4]    D skip            -> 286..290
CF_EPS = 290          # [128, 1] rmsnorm eps          -> 290..291
CF_TOT = 291

# bf16 const pack (cb16 [128, CB])
CB_IDN = 0            # [128, 128] identity           -> 0..128
CB_ONESC = 128        # [128, 1] ones column          -> 128..129
CB_ONESR = 129        # [1, 128] ones row (row 0)     -> 129..257
CB_TOT = 257

# gc [16, 2, 128, 8, 128] bf16 per-chunk host-computed decay tensors:
#   [:,0] G[j,h,i] = exp(cum_i-cum_j+ldt_j) masked (i>=j)
#   [:,1] EB[n,h,i] = exp(cum_i) replicated over partitions n


def _build_bass(cap=True):
    import concourse.bass as bass
    import concourse.mybir as mybir
    import concourse.tile as tile

    fp32 = mybir.dt.float32
    bf16 = mybir.dt.bfloat16
    AF = mybir.ActivationFunctionType
    OP = mybir.AluOpType

    nc = bass.Bass(num_devices=8)
    uT = nc.declare_dram_parameter("uT", [128, 8, L], bf16, isOutput=False)
    win = nc.declare_dram_parameter("win", [128, 8, 1280], bf16, isOutput=False)
    wo = nc.declare_dram_parameter("wo", [128, 8, 512], bf16, isOutput=False)
    cf = nc.declare_dram_parameter("cf", [128, CF_TOT], fp32, isOutput=False)
    cb16 = nc.declare_dram_parameter("cb16", [128, CB_TOT], bf16, isOutput=False)
    gcp = nc.declare_dram_parameter("gc", [16, 128, 2, 8, 128], bf16, isOutput=False)
    outT = nc.declare_dram_parameter("outT", [128, 4, L], fp32, isOutput=True)

    with tile.TileContext(nc) as tc:
        with (
            tc.tile_pool(name="const", bufs=1) as cp,
            tc.tile_pool(name="main", bufs=1) as mp,
            tc.tile_pool(name="dram", bufs=1, space="DRAM") as dp,
        ):
            c_f = cp.tile([128, CF_TOT], fp32)
            c_b = cp.tile([128, CB_TOT], bf16)
            wo_sb = cp.tile([128, 8, 512], bf16)
            nc.sync.dma_start(c_f[:], cf[:])
            nc.sync.dma_start(c_b[:], cb16[:])
            nc.sync.dma_start(wo_sb[:], wo[:])

            cw = c_f[:, CF_CW:CF_CW + 24].rearrange("p (j k) -> p j k", j=6)
            cb = c_f[:, CF_CB:CF_CB + 6]
            wpp = c_f[:, CF_WPP:CF_WPP + 128].rearrange("p (h c) -> p h c", h=8)
            dlpp = c_f[:, CF_DLPP:CF_DLPP + 128].rearrange("p (h c) -> p h c", h=8)
            dpp = c_f[:, CF_DPP:CF_DPP + 4]
            epsc = c_f[:, CF_EPS:CF_EPS + 1]
            idn = c_b[:, CB_IDN:CB_IDN + 128]
            onesc = c_b[:, CB_ONESC:CB_ONESC + 1]
            onesr = c_b[0:1, CB_ONESR:CB_ONESR + 128]

            # prime each compute engine on the const DMA lanes
            prim = cp.tile([128, 24], fp32)
            nc.vector.tensor_copy(prim[:, 0:1], c_f[:, 0:1])
            nc.vector.tensor_copy(prim[:, 1:2], c_b[:, 0:1])
            nc.gpsimd.tensor_copy(prim[:, 2:3], c_f[:, 0:1])
            nc.gpsimd.tensor_copy(prim[:, 3:4], c_b[:, 0:1])
            nc.scalar.copy(prim[:, 4:5], c_f[:, 0:1])
            nc.scalar.copy(prim[:, 5:6], c_b[:, 0:1])

            H_sb = mp.tile([128, 8, 64], fp32)     # SSD state per head [n, p]
            nc.vector.memset(H_sb[:], 0.0)
            gin8 = mp.tile([128, 8, L], bf16)      # gathered yz (full channels)

            # ---------------- phases 1+2: activations pool ----------------
            ap_ctx = tc.tile_pool(name="actp", bufs=1)
            ap_ = ap_ctx.__enter__()
            z_sb = ap_.tile([128, 4, L], bf16)     # silu(z), channel-major
            xbc = ap_.tile([128, 6, L], bf16)      # conv+silu out: x 0..3, B=4, C=5
            y_sb = ap_.tile([128, 4, L], bf16)     # y then yz, channel-major

            # ---------------- phase 1: in_proj + conv ----------------
            with (
                tc.tile_pool(name="p1", bufs=1) as p1,
                tc.tile_pool(name="pp1", bufs=2, space="PSUM") as pp1,
            ):
                win_sb = p1.tile([128, 8, 1280], bf16)
                nc.sync.dma_start(win_sb[:], win[:])
                uT_sb = p1.tile([128, 8, L], bf16)
                nc.sync.dma_start(uT_sb[:], uT[:])
                raw = p1.tile([128, 6, L + 4], bf16)
                nc.vector.memset(raw[:, :, 0:3], 0.0)

                _srcs = [c_f[:, 0:1], c_b[:, 0:1], wo_sb[:, 0, 0:1],
                         win_sb[:, 0, 0:1], uT_sb[:, 0, 0:1]]
                for _i, _src in enumerate(_srcs):
                    nc.vector.tensor_copy(prim[:, 4 * _i:4 * _i + 1], _src)
                    nc.gpsimd.tensor_copy(prim[:, 4 * _i + 1:4 * _i + 2], _src)
                    nc.scalar.copy(prim[:, 4 * _i + 2:4 * _i + 3], _src)

                for ct in (8, 9, 4, 5, 6, 7, 0, 1, 2, 3):
                    pss = [pp1.tile([128, 512], fp32, name=f"ps_in{tt}",
                                    tag=f"ps{tt}", bufs=2)
                           for tt in range(4)]
                    for kt in range(8):
                        for tt in range(4):
                            nc.tensor.matmul(
                                pss[tt][:], win_sb[:, kt, ct * 128:(ct + 1) * 128],
                                uT_sb[:, kt, tt * 512:(tt + 1) * 512],
                                start=(kt == 0), stop=(kt == 7))
                    for tt in range(4):
                        if ct < 4:
                            nc.scalar.activation(
                                z_sb[:, ct, tt * 512:(tt + 1) * 512], pss[tt][:], AF.Silu)
                        else:
                            nc.scalar.copy(
                                raw[:, ct - 4, 3 + tt * 512: 3 + (tt + 1) * 512],
                                pss[tt][:])
                    if ct >= 4:
                        j = ct - 4
                        a0 = p1.tile([128, L], fp32, name="cacc", tag="cacc", bufs=3)
                        nc.scalar.mul(a0[:], raw[:, j, 0:L], cw[:, j, 0:1])
                        a1 = p1.tile([128, L], fp32, name="cacc1", tag="cacc", bufs=3)
                        nc.vector.scalar_tensor_tensor(
                            a1[:], raw[:, j, 1:1 + L], cw[:, j, 1:2], a0[:],
                            op0=OP.mult, op1=OP.add)
                        a2 = p1.tile([128, L], fp32, name="cacc2", tag="cacc", bufs=3)
                        nc.vector.scalar_tensor_tensor(
                            a2[:], raw[:, j, 2:2 + L], cw[:, j, 2:3], a1[:],
                            op0=OP.mult, op1=OP.add)
                        a3 = p1.tile([128, L], fp32, name="cacc3", tag="cacc", bufs=3)
                        nc.vector.scalar_tensor_tensor(
                            a3[:], raw[:, j, 3:3 + L], cw[:, j, 3:4], a2[:],
                            op0=OP.mult, op1=OP.add)
                        nc.scalar.activation(xbc[:, j, :], a3[:], AF.Silu,
                                             bias=cb[:, j:j + 1])

            # ---------------- phase 2: chunked SSD + half gathers ----------------
            def phase3_half(hf):
                """norm + out_proj for tokens [hf*1024, hf*1024+1024)."""
                hoff = hf * 1024
                yzsq = p3.tile([128, 8, 1024], bf16, name="yzsq", tag="yzsq", bufs=1)
                nc.scalar.activation(yzsq[:, 0:4, :],
                                     gin8[:, 0:4, hoff:hoff + 1024], AF.Square)
                nc.scalar.activation(yzsq[:, 4:8, :],
                                     gin8[:, 4:8, hoff:hoff + 1024], AF.Square)

                sq = p3.tile([1, 1024], fp32, name="sq", tag="sq", bufs=2)
                for tt in range(2):
                    ss_ps = pp3.tile([1, 512], fp32, name="ssps", tag="ssps", bufs=1)
                    for kt in range(8):
                        nc.tensor.matmul(ss_ps[:], onesc,
                                         yzsq[:, kt, tt * 512:(tt + 1) * 512],
                                         start=(kt == 0), stop=(kt == 7))
                    nc.scalar.activation(sq[:, tt * 512:(tt + 1) * 512], ss_ps[:],
                                         AF.Sqrt, bias=epsc[0:1, :], scale=1.0 / 1024.0)
                rsq = p3.tile([1, 1024], fp32, name="rsq", tag="rsq", bufs=2)
                nc.vector.reciprocal(rsq[:], sq[:])
                rsqb = p3.tile([1, 1024], bf16, name="rsqb", tag="rsqb", bufs=2)
                nc.scalar.copy(rsqb[:], rsq[:])
                rb_sb = p3.tile([128, 2, 512], fp32, name="rb", tag="rb", bufs=2)
                for tt in range(2):
                    rb_ps = pp3.tile([128, 512], fp32, name="rbps", tag="rbps", bufs=1)
                    nc.tensor.matmul(rb_ps[:], onesr,
                                     rsqb[:, tt * 512:(tt + 1) * 512],
                                     start=True, stop=True)
                    nc.scalar.copy(rb_sb[:, tt, :], rb_ps[:])
                out_sb = p3.tile([128, 4, 1024], fp32, name="osb", tag="osb", bufs=1)
                for dt_ in range(4):
                    for tt in range(2):
                        op_ps = pp3.tile([128, 512], fp32, name="opps", tag="opps", bufs=2)
                        for kt in range(8):
                            nc.tensor.matmul(
                                op_ps[:], wo_sb[:, kt, dt_ * 128:(dt_ + 1) * 128],
                                gin8[:, kt, hoff + tt * 512: hoff + (tt + 1) * 512],
                                start=(kt == 0), stop=(kt == 7))
                        nc.vector.tensor_tensor(
                            out_sb[:, dt_, tt * 512:(tt + 1) * 512], op_ps[:],
                            rb_sb[:, tt, :], op=OP.mult)
                nc.gpsimd.dma_start(outT[:, :, hoff:hoff + 1024], out_sb[:])

            with (
                tc.tile_pool(name="p2", bufs=1) as p2,
                tc.tile_pool(name="pp2", bufs=1, space="PSUM") as pp2,
            ):
                p3_ctx = tc.tile_pool(name="p3", bufs=1)
                p3 = p3_ctx.__enter__()
                pp3_ctx = tc.tile_pool(name="pp3", bufs=1, space="PSUM")
                pp3 = pp3_ctx.__enter__()
                yzbs = [dp.tile([4, 128, 1024], bf16, name=f"yzb{q}") for q in range(2)]
                gaths = [dp.tile([8, 128, 1024], bf16, name=f"gath{q}") for q in range(2)]
                for c in range(NCH):
                    veng = nc.gpsimd if c < 8 else nc.vector
                    tok = slice(c * Q, (c + 1) * Q)
                    gch = p2.tile([128, 2, 8, 128], bf16, name="gch", tag="gch", bufs=2)
                    nc.scalar.dma_start(gch[:], gcp[c])
                    g_sb = gch[:, 0]
                    eb_sb = gch[:, 1]

                    xt_sb = p2.tile([128, 512], bf16, name="xt", tag="xt", bufs=2)
                    for ct in range(4):
                        tp = pp2.tile([128, 128], bf16, name="tp", tag="ps128", bufs=3)
                        nc.tensor.transpose(tp[:], xbc[:, ct, tok], idn)
                        nc.scalar.copy(xt_sb[:, ct * 128:(ct + 1) * 128], tp[:])
                    btp = pp2.tile([128, 128], bf16, name="btp", tag="ps128", bufs=3)
                    nc.tensor.transpose(btp[:], xbc[:, 4, tok], idn)
                    bt_sb = p2.tile([128, 128], bf16, name="bt", tag="bt", bufs=2)
                    nc.scalar.copy(bt_sb[:], btp[:])

                    mt_ps = pp2.tile([128, 128], fp32, name="mt", tag="ps128", bufs=3)
                    nc.tensor.matmul(mt_ps[:], xbc[:, 4, tok], xbc[:, 5, tok],
                                     start=True, stop=True)

                    bw_all = p2.tile([128, 8, 128], bf16, name="bwall", tag="bwall", bufs=2)
                    veng.tensor_tensor(
                        bw_all[:],
                        bt_sb[:].unsqueeze(1).broadcast_to([128, 8, 128]),
                        wpp[:, :, c].unsqueeze(2).broadcast_to([128, 8, 128]),
                        op=OP.mult)

                    hbf = p2.tile([128, 8, 64], bf16, name="hbf", tag="hbf", bufs=2)
                    nc.scalar.copy(hbf[:], H_sb[:])

                    m2 = p2.tile([128, 8, 128], bf16, name="m2", tag="m2", bufs=2)
                    nc.vector.tensor_tensor(
                        m2[:], mt_ps[:].unsqueeze(1).broadcast_to([128, 8, 128]),
                        g_sb, op=OP.mult)
                    ctl = p2.tile([128, 8, 128], bf16, name="ctl", tag="ctl", bufs=2)
                    veng.tensor_tensor(
                        ctl[:], eb_sb,
                        xbc[:, 5, tok].unsqueeze(1).broadcast_to([128, 8, 128]),
                        op=OP.mult)

                    s_ps = pp2.tile([128, 8, 64], fp32, name="sps", tag="sps", bufs=1)
                    for h in range(HL):
                        pr = h // 2
                        off = (h % 2) * 64
                        if h % 2 == 0:
                            y_ps = pp2.tile([128, 128], fp32, name="yps",
                                            tag="ps128", bufs=3)
                        nc.tensor.matmul(
                            y_ps[off:off + 64, :],
                            xt_sb[:, pr * 128 + off: pr * 128 + off + 64], m2[:, h, :],
                            start=True, stop=False)
                        nc.tensor.matmul(y_ps[off:off + 64, :], hbf[:, h, :],
                                         ctl[:, h, :], start=False, stop=True)
                        nc.tensor.matmul(
                            s_ps[:, h, :], bw_all[:, h, :],
                            xt_sb[:, pr * 128 + off: pr * 128 + off + 64],
                            start=True, stop=True)
                        if h % 2 == 1:
                            nc.vector.scalar_tensor_tensor(
                                y_sb[:, pr, tok], xbc[:, pr, tok], dpp[:, pr:pr + 1],
                                y_ps[:], op0=OP.mult, op1=OP.add)

                    hdl = p2.tile([128, 8, 64], fp32, name="hdl", tag="hdl", bufs=2)
                    veng.tensor_tensor(
                        hdl[:], H_sb[:],
                        dlpp[:, :, c].unsqueeze(2).broadcast_to([128, 8, 64]),
                        op=OP.mult)
                    nc.vector.tensor_tensor(H_sb[:], hdl[:], s_ps[:], op=OP.add)

                    # half gather: yz for tokens [hf*1024, hf*1024+1024) when ready
                    if c % 8 == 7:
                        hf = c // 8
                        qs = slice(hf * 1024, (hf + 1) * 1024)
                        nc.vector.tensor_tensor(
                            y_sb[:, :, qs], y_sb[:, :, qs], z_sb[:, :, qs], op=OP.mult)
                        nc.sync.dma_start(yzbs[hf].rearrange("c p t -> p c t"),
                                          y_sb[:, :, qs])
                        nc.gpsimd.collective_compute(
                            "AllGather", OP.bypass,
                            replica_groups=[[0, 1], [2, 3], [4, 5], [6, 7]],
                            ins=[yzbs[hf].opt()], outs=[gaths[hf].opt()])
                        nc.sync.dma_start(gin8[:, :, qs],
                                          gaths[hf].rearrange("c p t -> p c t"))
                        phase3_half(hf)

                p3_ctx.__exit__(None, None, None)
                pp3_ctx.__exit__(None, None, None)
            ap_ctx.__exit__(None, None, None)
    if cap:
        _cap_waits(nc)
    return nc


def _cap_waits(nc):
    """Walrus rejects instructions with too many semaphore waits (2 for PE
    matmuls, 1 for everything else). Hoist excess waits onto same-engine
    NoOps inserted immediately before the offender: the engine sequencer
    executes the NoOp's waits first, which is semantically identical and
    deadlock-free (no intervening same-engine instruction)."""
    import concourse.mybir as mb
    nid = [0]
    for f in nc.m.functions:
        for bb in f.blocks:
            newlist = []
            changed = False
            for ins in bb.instructions:
                si = ins.sync_info
                waits = list(si.on_wait) if si is not None else []
                cap = 1
                if len(waits) > cap:
                    excess, keep = waits[:-cap], waits[-cap:]
                    for w in excess:
                        nop = mb.InstNoOp(name=f"waitnop_{nid[0]}")
                        nid[0] += 1
                        nop.engine = ins.engine
                        nop.sync_info = mb.SyncInfo(on_wait=[w], on_update=[])
                        newlist.append(nop)
                    si.on_wait = keep
                    changed = True
                newlist.append(ins)
            if changed:
                bb.instructions = newlist


_NC_CACHE = None


def _get_nc():
    global _NC_CACHE
    if _NC_CACHE is None:
        _NC_CACHE = _build_bass()
    return _NC_CACHE


def _softplus(x):
    return np.logaddexp(0.0, x)


def _host_prep(u, in_proj_w, h_proj_w, conv_w, conv_b, dt_bias, gamma,
               A_log, D, norm_w, out_proj_w):
    """Returns per-core in_maps for the device kernel."""
    u = np.asarray(u, F32)
    in_proj_w = np.asarray(in_proj_w, F32)
    h_proj_w = np.asarray(h_proj_w, F32)

    # --- dt path on host (tiny) ---
    W1 = h_proj_w[:, :1024]
    W2 = h_proj_w[:, 1024:]
    W_dt = in_proj_w[2304:, :]
    p40 = u @ W1.T                                    # [B,L,40]
    dt_all = u @ W_dt.T                               # [B,L,32]
    denom = np.arange(1, L + 1, dtype=np.float64)[None, :, None]
    cmean = (np.cumsum(p40.astype(np.float64), 1) / denom).astype(F32)
    hb = p40 - cmean + dt_all @ W2.T
    hsc = hb[..., :28]
    sbias = hb[..., 28:]
    sel = np.argsort(-hsc, axis=-1, kind="stable")[..., :12]
    ids = np.concatenate(
        [sel, np.broadcast_to(np.arange(28, 32), (B, L, 4))], -1)
    dtg = np.take_along_axis(dt_all, ids, -1)
    sb = np.asarray(gamma, F32) * np.concatenate(
        [sbias, np.zeros((B, L, 4), F32)], -1)
    dt = _softplus(dtg + np.asarray(dt_bias, F32) + sb)     # [B,L,16]

    A = -np.exp(np.asarray(A_log, F32))
    a = (dt * A[None, None, :]).reshape(B, NCH, Q, H)
    dtr = dt.reshape(B, NCH, Q, H)
    cum = np.cumsum(a, axis=2)                        # [B,NCH,Q,H]
    ldt = np.log(np.maximum(dtr, 1e-38))
    w_ = np.exp(cum[:, :, -1:, :] - cum) * dtr        # [B,NCH,Q,H]
    dl = np.exp(cum[:, :, -1, :])                     # [B,NCH,H]
    ecum = np.exp(cum)                                # [B,NCH,Q,H]

    Dv = np.asarray(D, F32)
    nwv = np.asarray(norm_w, F32)
    wop = np.asarray(out_proj_w, F32) * nwv[None, :]  # fold norm_w
    cwv = np.asarray(conv_w, F32)
    cbv = np.asarray(conv_b, F32)

    in_maps = []
    for c_ in range(8):
        b, g = c_ // 2, c_ % 2
        hs = slice(8 * g, 8 * g + 8)

        uTc = np.ascontiguousarray(u[b].T).reshape(8, 128, L).transpose(1, 0, 2)
        uTc = np.ascontiguousarray(uTc).astype(BF16)  # [128p, 8kt, L]

        rows = np.concatenate([
            np.arange(512 * g, 512 * g + 512),            # z
            np.arange(1024 + 512 * g, 1024 + 512 * g + 512),  # x
            np.arange(2048, 2304),                        # B, C
        ])
        winc = in_proj_w[rows, :].T                       # [1024, 1280]
        winc = winc.reshape(8, 128, 1280).transpose(1, 0, 2)
        winc = np.ascontiguousarray(winc).astype(BF16)    # [128p, 8kt, 1280]

        woc = wop[512 * g:512 * g + 512, :].T             # [1024ch, 512dout]
        woc = woc.reshape(8, 128, 512).transpose(1, 0, 2)
        woc = np.ascontiguousarray(woc).astype(BF16)

        cfc = np.zeros((128, CF_TOT), F32)
        # conv weights/bias: conv channels for chtiles j=0..3 (x), 4 (B), 5 (C)
        cch = np.concatenate([
            np.arange(512 * g, 512 * g + 512),            # x conv channels
            np.arange(1024, 1152),                        # B
            np.arange(1152, 1280),                        # C
        ]).reshape(6, 128)
        for j in range(6):
            cfc[:, CF_CW + j * 4:CF_CW + (j + 1) * 4] = cwv[cch[j]]
            cfc[:, CF_CB + j] = cbv[cch[j]]
        cfc[:, CF_WPP:CF_WPP + 128] = (
            w_[b, :, :, hs].transpose(1, 2, 0).reshape(128, 128))
        # ^ w_[b, c, j, h] -> [j, h, c] -> [128, 8*16]
        cfc[:, CF_DLPP:CF_DLPP + 128] = np.broadcast_to(
            dl[b, :, hs].T.reshape(1, 128), (128, 128))
        # ^ dl[b, c, h] -> [h, c] -> flat 128, replicated over partitions
        for ct in range(4):
            cfc[:, CF_DPP + ct] = np.repeat(Dv[8 * g + 2 * ct: 8 * g + 2 * ct + 2], 64)
        cfc[:, CF_EPS] = EPS

        cbc = np.zeros((128, CB_TOT), F32)
        cbc[:, CB_IDN:CB_IDN + 128] = np.eye(128, dtype=F32)
        cbc[:, CB_ONESC] = 1.0
        cbc[0, CB_ONESR:CB_ONESR + 128] = 1.0

        # G[c, j, h, i] = exp(cum_i - cum_j + ldt_j) masked to i>=j
        # EB[c, n, h, i] = exp(cum_i) for every n
        cumc = cum[b, :, :, hs]                       # [c, j(i), h]
        ldtc = ldt[b, :, :, hs]
        E = (cumc[:, None, :, :] - cumc[:, :, None, :] + ldtc[:, :, None, :])
        # E[c, j, i, h]
        np.minimum(E, 0.0, out=E)
        G = np.exp(E, out=E)
        G *= (np.arange(Q)[None, :, None, None] <= np.arange(Q)[None, None, :, None])
        gc_arr = np.empty((16, 128, 2, 8, 128), BF16)
        gc_arr[:, :, 0] = G.transpose(0, 1, 3, 2)     # [c, j, h, i]
        gc_arr[:, :, 1] = np.broadcast_to(
            ecum[b, :, :, hs].transpose(0, 2, 1)[:, None, :, :], (16, 128, 8, 128))

        in_maps.append(dict(uT=uTc, win=winc, wo=woc, cf=cfc,
                            cb16=cbc.astype(BF16), gc=gc_arr))
    return in_maps


def _assemble(results):
    out = np.empty((B, L, DM), F32)
    for c_ in range(8):
        b, g = c_ // 2, c_ % 2
        oT = np.asarray(results[c_]["outT"], F32)     # [128, 4, L]
        # out[b, t, 512g + dt*128 + p] = oT[p, dt, t]
        out[b, :, 512 * g:512 * g + 512] = (
            oT.transpose(2, 1, 0).reshape(L, 512))
    return out


def _numpy_fallback(u, in_proj_w, h_proj_w, conv_w, conv_b, dt_bias, gamma,
                    A_log, D, norm_w, out_proj_w):
    u = np.asarray(u, F32)
    in_proj_w = np.asarray(in_proj_w, F32)
    zxbcdt = u @ in_proj_w.T
    dt_all = zxbcdt[..., -32:]
    denom = np.arange(1, L + 1, dtype=np.float64)[None, :, None]
    sr = (u - np.cumsum(u.astype(np.float64), 1) / denom).astype(F32)
    udt = np.concatenate([sr, dt_all], -1)
    hb = udt @ np.asarray(h_proj_w, F32).T
    hsc = hb[..., :28]
    sbias = hb[..., 28:]
    sel = np.argsort(-hsc, axis=-1, kind="stable")[..., :12]
    ids = np.concatenate([sel, np.broadcast_to(np.arange(28, 32), (B, L, 4))], -1)
    dtv = np.take_along_axis(dt_all, ids, -1)
    sb = np.asarray(gamma, F32) * np.concatenate(
        [sbias, np.zeros((B, L, 4), F32)], -1)
    dtv = _softplus(dtv + np.asarray(dt_bias, F32) + sb)
    z = zxbcdt[..., :1024]
    xBC = zxbcdt[..., 1024:2304]
    cw = np.asarray(conv_w, F32)
    conv = np.zeros_like(xBC)
    for k in range(4):
        shift = 3 - k
        src = xBC[:, :L - shift] if shift else xBC
        conv[:, shift:] += src * cw[None, None, :, k]
    xBC = conv + np.asarray(conv_b, F32)[None, None, :]
    xBC = xBC / (1.0 + np.exp(-xBC))
    x = xBC[..., :1024].reshape(B, L, H, P)
    Bm = xBC[..., 1024:1152]
    Cm = xBC[..., 1152:]
    A = -np.exp(np.asarray(A_log, F32))
    decay = np.exp(dtv * A[None, None, :])
    dtx = dtv[..., None] * x
    state = np.zeros((B, H, P, N), F32)
    y = np.empty((B, L, H, P), F32)
    for t in range(L):
        state *= decay[:, t, :, None, None]
        state += dtx[:, t, :, :, None] * Bm[:, t, None, None, :]
        y[:, t] = np.einsum("bhpn,bn->bhp", state, Cm[:, t])
    y = y + np.asarray(D, F32)[None, None, :, None] * x
    y = y.reshape(B, L, 1024)
    yz = y * (z / (1.0 + np.exp(-z)))
    yn = yz / np.sqrt(np.mean(yz * yz, -1, keepdims=True) + EPS) \
        * np.asarray(norm_w, F32)
    return (yn @ np.asarray(out_proj_w, F32).T).astype(F32)


def kernel(u, in_proj_w, h_proj_w, conv_w, conv_b, dt_bias, gamma,
           A_log, D, norm_w, out_proj_w):
    args = (u, in_proj_w, h_proj_w, conv_w, conv_b, dt_bias, gamma,
            A_log, D, norm_w, out_proj_w)
    try:
        from concourse.bass_utils import run_bass_kernel_spmd
        in_maps = _host_prep(*args)
        nc = _get_nc()
        res = run_bass_kernel_spmd(nc, in_maps, list(range(8)))
        return _assemble(res.results)
    except Exception as e:
        import sys, traceback
        traceback.print_exc()
        print(f"[kernel] device path failed ({e!r}); numpy fallback",
              file=sys.stderr)
        return _numpy_fallback(*args)
